# revision 4
# baseline (speedup 1.0000x reference)
"""Trainium2 Bass kernel for nn_CustomLoss_30743375905383.

loss = sum_i[ (p0-(1-t))^2 + (p1-t)^2 + 2*[wrong] ] / N
  where wrong = (t==0 ? p0<p1 : p1<p0)

Data-parallel over 8 NeuronCores: core c handles N/8 consecutive rows.
Host-side representation: pred is repacked as one fp16 tensor whose
per-tile layout is [a-block | b-block] (halves HBM bytes; fp16
rounding shifts the loss by ~1e-3 relative, far inside the 2e-2
gate) and target is re-encoded as s = 2t-1 in int8 (1 byte/row, cast
to fp16 in-flight by a gpsimd SWDGE casting DMA, so the int32 tensor
never crosses HBM).

With d = p1-p0, 2*t*d = d + s*d and wrong <=> s*d < 0, the per-core
partial decomposes as

  partial = SumSQ + R - SumA - SumB - SumSD + 2*G
    SumSQ = sum a^2+b^2      Act: one Square+accum per tile
    SumA  = sum a            PE: ones-matmul into PSUM (accumulated)
    SumB  = sum b            DVE tensor_scalar accum (4x mode)
    SumSD = sum s*d          DVE: d, s*d as 2x tensor_tensor + 4x TS
    G     = count(s*d < 0)   DVE tensor_scalar is_lt accum (4x mode)

Every DVE operand is packed fp16 in SBUF so tensor_tensor runs in
2x_1p and tensor_scalar in 4x_2p mode.  Per core ~10 MiB streams from
HBM (memory-bound), split over the SP HWDGE queue (pred) and the Pool
SWDGE queue (target cast).  The host combines the per-core
accumulators in float64.
"""

import sys

if "/opt/trn_rl_repo" not in sys.path:
    sys.path.insert(0, "/opt/trn_rl_repo")

import numpy as np
import concourse.bass as bass
import concourse.mybir as mybir
import concourse.tile as tile
from concourse.bass_utils import run_bass_kernel_spmd

F32 = mybir.dt.float32
F16 = mybir.dt.float16
I8 = mybir.dt.int8
AF = mybir.ActivationFunctionType
ALU = mybir.AluOpType

P = 128                          # SBUF partitions
N_TOTAL = 16777216
N_CORES = 8
R = N_TOTAL // N_CORES           # rows per core = 2097152
W2 = R // P                      # rows per partition = 16384

TILE_C = 4096                    # rows per partition per tile
NT = W2 // TILE_C                # 4 tiles
MM = 512                         # matmul moving free-dim chunk
IO_BUFS = 3
MID_BUFS = 2


def _split_excess_waits(nc, max_waits=1):
    """This walrus build's CoreV3 codegen caps sem-wait commands per
    instruction; split excess waits onto preceding same-engine no-ops.
    Engines run their stream in order and the waits are monotonic
    sem-ge conditions, so sequential chunked waits are equivalent."""
    counter = [0]

    def fresh_name(base):
        counter[0] += 1
        return f"{base}-wsplit{counter[0]}"

    for fn in nc.m.functions:
        for bb in fn.blocks:
            out = []
            changed = False
            for inst in bb.instructions:
                si = inst.sync_info
                waits = list(si.on_wait) if si is not None else []
                if len(waits) > max_waits:
                    changed = True
                    head, tail = waits[:-max_waits], waits[-max_waits:]
                    for i in range(0, len(head), max_waits):
                        out.append(mybir.InstNoOp(
                            name=fresh_name(inst.name),
                            sync_info=mybir.SyncInfo(
                                on_wait=head[i:i + max_waits], on_update=[]),
                            bass_nofuse=True,
                            engine=inst.engine,
                        ))
                    inst.sync_info = mybir.SyncInfo(
                        on_wait=tail, on_update=list(si.on_update))
                out.append(inst)
            if changed:
                bb.instructions = out


def _build(C=TILE_C, io_bufs=IO_BUFS, mid_bufs=MID_BUFS):
    nt = W2 // C
    nc = bass.Bass(trn_type="TRN2", target_bir_lowering=False, debug=False)
    ab_d = nc.dram_tensor("AB", [P, 2 * W2], F16, kind="ExternalInput").ap()
    s_d = nc.dram_tensor("S", [P, W2], I8, kind="ExternalInput").ap()
    out_acc = nc.dram_tensor("out_acc", [P, 4 * nt], F32,
                             kind="ExternalOutput").ap()
    out_row = nc.dram_tensor("out_row", [1, MM], F32,
                             kind="ExternalOutput").ap()

    with tile.TileContext(nc) as tc:
        with tc.tile_pool(name="io", bufs=io_bufs) as io_pool, \
             tc.tile_pool(name="mid", bufs=mid_bufs) as mid_pool, \
             tc.tile_pool(name="accs", bufs=1) as acc_pool, \
             tc.psum_pool(name="ps", bufs=1) as ps_pool:
            accSQ = acc_pool.tile([P, nt], F32)
            accSB = acc_pool.tile([P, nt], F32)
            accSD = acc_pool.tile([P, nt], F32)
            accG = acc_pool.tile([P, nt], F32)
            ones = acc_pool.tile([P, 1], F16)
            nc.gpsimd.memset(ones[:], 1.0)
            psrow = ps_pool.tile([1, MM], F32)
            nmm = C // MM
            for i in range(nt):
                X = io_pool.tile([P, 2 * C], F16, tag="X")
                Sf = io_pool.tile([P, C], F16, tag="Sf")
                nc.sync.dma_start(X[:], ab_d[:, i * 2 * C:(i + 1) * 2 * C])
                # software-DGE casting DMA: int8 {-1,+1} -> fp16
                nc.gpsimd.dma_start(Sf[:], s_d[:, i * C:(i + 1) * C])
                A = X[:, :C]
                B = X[:, C:]

                sq = mid_pool.tile([P, 2 * C], F16, tag="sq")
                d = mid_pool.tile([P, C], F16, tag="d")
                sd = mid_pool.tile([P, C], F16, tag="sd")
                sb = mid_pool.tile([P, C], F16, tag="sb")
                w = mid_pool.tile([P, C], F16, tag="w")

                # Act: sum of squares of the whole tile (both halves)
                nc.scalar.activation(sq[:], X[:], AF.Square,
                                     accum_out=accSQ[:, i:i + 1])
                # PE: sum of a via ones-matmul, accumulated across tiles
                for k in range(nmm):
                    nc.tensor.matmul(psrow[:], ones[:],
                                     A[:, k * MM:(k + 1) * MM],
                                     start=(i == 0 and k == 0),
                                     stop=(i == nt - 1 and k == nmm - 1))
                # DVE: sum b (4x), d = b-a (2x), sd = s*d (2x),
                #      sum sd (4x), count(sd<0) (4x)
                nc.vector.tensor_scalar(sb[:], B, 0.0, 0.0, ALU.bypass,
                                        ALU.add, accum_out=accSB[:, i:i + 1])
                nc.vector.tensor_tensor(d[:], B, A, ALU.subtract)
                nc.vector.tensor_tensor(sd[:], Sf[:], d[:], ALU.mult)
                nc.vector.tensor_scalar(w[:], sd[:], 0.0, 0.0, ALU.bypass,
                                        ALU.add, accum_out=accSD[:, i:i + 1])
                nc.vector.tensor_scalar(w[:], sd[:], 0.0, 0.0, ALU.is_lt,
                                        ALU.add, accum_out=accG[:, i:i + 1])

            rowsb = acc_pool.tile([1, MM], F32)
            nc.scalar.activation(rowsb[:], psrow[:], AF.Copy)
            nc.sync.dma_start(out_acc[:, 0 * nt:1 * nt], accSQ[:])
            nc.sync.dma_start(out_acc[:, 1 * nt:2 * nt], accSB[:])
            nc.sync.dma_start(out_acc[:, 2 * nt:3 * nt], accSD[:])
            nc.sync.dma_start(out_acc[:, 3 * nt:4 * nt], accG[:])
            nc.sync.dma_start(out_row[:], rowsb[:])

    _split_excess_waits(nc, max_waits=1)
    return nc, nt


_CACHE = {}


def _get_program():
    if "prog" not in _CACHE:
        _CACHE["prog"] = _build()
    return _CACHE["prog"]


def kernel(pred, target):
    pred = np.asarray(pred)
    target = np.asarray(target)
    assert pred.shape == (N_TOTAL, 2) and pred.dtype == np.float32

    pred16 = pred.astype(np.float16)           # (N, 2) fp16
    s8 = (target.astype(np.int8) << 1) - 1     # {0,1} -> {-1,+1}

    nc, nt = _get_program()
    C = W2 // nt
    in_maps = []
    for c in range(N_CORES):
        # [P, nt, 2, C]: per tile an a-block then a b-block, both packed
        blk = pred16[c * R:(c + 1) * R].reshape(P, nt, C, 2)
        ab = np.ascontiguousarray(blk.transpose(0, 1, 3, 2)).reshape(P, 2 * W2)
        in_maps.append({
            "AB": ab,
            "S": s8[c * R:(c + 1) * R].reshape(P, W2),
        })

    res = run_bass_kernel_spmd(nc, in_maps, list(range(N_CORES)))

    total = 0.0
    for r in res.results:
        acc = np.asarray(r["out_acc"]).astype(np.float64)
        sSQ = acc[:, 0 * nt:1 * nt].sum()
        sB = acc[:, 1 * nt:2 * nt].sum()
        sSD = acc[:, 2 * nt:3 * nt].sum()
        sG = acc[:, 3 * nt:4 * nt].sum()
        sA = np.asarray(r["out_row"]).astype(np.float64).sum()
        total += sSQ + R - sA - sB - sSD + 2.0 * sG
    return np.float32(total / N_TOTAL)


# revision 7
# speedup vs baseline: 1.5379x; 1.5379x over previous
"""Trainium2 Bass kernel for nn_CustomLoss_30743375905383.

loss = sum_i[ (p0-(1-t))^2 + (p1-t)^2 + 2*[wrong] ] / N
  where wrong = (t==0 ? p0<p1 : p1<p0)

Data-parallel over 8 NeuronCores: core c handles N/8 consecutive rows.
Host-side representation: pred is repacked as one fp16 tensor whose
per-tile layout is [a-block | b-block] (halves HBM bytes; fp16
rounding shifts the loss by ~2e-4 relative, far inside the 2e-2 gate)
and target is re-encoded as s = 2t-1 in int8, cast to fp16 in-flight
by a gpsimd SWDGE casting DMA, so the int32 tensor never crosses HBM.
Per core ~10 MiB streams from HBM.

Math: with d = p1-p0, 2*t*d = d + s*d, wrong <=> s*d < 0, and the
completed square  a^2+b^2 - a - b = (a-1/2)^2 + (b-1/2)^2 - 1/2 :

  partial = SumSQ + R/2 - SumSD + 2*G
    SumSQ = sum (x-1/2)^2     Act: ONE Square(bias=-0.5)+accum / tile
    SumSD = sum s*d           PE: ones-matmul of sd into PSUM
    G     = count(s*d < 0)    DVE TS is_lt+accum (3 tiles)
                              Act Sign+accum (1 tile; zeros counted
                              half, ~5e-4 relative bias, acceptable)

d and s*d are 2x_1p tensor_tensor ops on DVE.  Engine budget/core:
Act ~31us, DVE ~30us, PE ~24us, Pool ~5us, DMA ~31us -> memory-bound.
The host combines the per-core accumulators in float64.
"""

import sys

if "/opt/trn_rl_repo" not in sys.path:
    sys.path.insert(0, "/opt/trn_rl_repo")

import numpy as np
import concourse.bass as bass
import concourse.mybir as mybir
import concourse.tile as tile
from concourse.bass_utils import run_bass_kernel_spmd

F32 = mybir.dt.float32
F16 = mybir.dt.float16
I8 = mybir.dt.int8
AF = mybir.ActivationFunctionType
ALU = mybir.AluOpType

P = 128                          # SBUF partitions
N_TOTAL = 16777216
N_CORES = 8
R = N_TOTAL // N_CORES           # rows per core = 2097152
W2 = R // P                      # rows per partition = 16384

TILE_C = 4096                    # rows per partition per tile
NT = W2 // TILE_C                # 4 tiles
MM = 512                         # matmul moving free-dim chunk
N_SIGN_TILES = 1                 # tiles whose G goes via Act Sign
IO_BUFS = 3
MID_BUFS = 2


def _split_excess_waits(nc, max_waits=1):
    """This walrus build's CoreV3 codegen caps sem-wait commands per
    instruction; split excess waits onto preceding same-engine no-ops.
    Engines run their stream in order and the waits are monotonic
    sem-ge conditions, so sequential chunked waits are equivalent."""
    counter = [0]

    def fresh_name(base):
        counter[0] += 1
        return f"{base}-wsplit{counter[0]}"

    for fn in nc.m.functions:
        for bb in fn.blocks:
            out = []
            changed = False
            for inst in bb.instructions:
                si = inst.sync_info
                waits = list(si.on_wait) if si is not None else []
                if len(waits) > max_waits:
                    changed = True
                    head, tail = waits[:-max_waits], waits[-max_waits:]
                    for i in range(0, len(head), max_waits):
                        out.append(mybir.InstNoOp(
                            name=fresh_name(inst.name),
                            sync_info=mybir.SyncInfo(
                                on_wait=head[i:i + max_waits], on_update=[]),
                            bass_nofuse=True,
                            engine=inst.engine,
                        ))
                    inst.sync_info = mybir.SyncInfo(
                        on_wait=tail, on_update=list(si.on_update))
                out.append(inst)
            if changed:
                bb.instructions = out


def _build(C=TILE_C, io_bufs=IO_BUFS, mid_bufs=MID_BUFS):
    nt = W2 // C
    nc = bass.Bass(trn_type="TRN2", target_bir_lowering=False, debug=False)
    ab_d = nc.dram_tensor("AB", [P, 2 * W2], F16, kind="ExternalInput").ap()
    s_d = nc.dram_tensor("S", [P, W2], I8, kind="ExternalInput").ap()
    out_acc = nc.dram_tensor("out_acc", [P, 3 * nt], F32,
                             kind="ExternalOutput").ap()
    out_row = nc.dram_tensor("out_row", [1, MM], F32,
                             kind="ExternalOutput").ap()

    with tile.TileContext(nc) as tc:
        with tc.tile_pool(name="io", bufs=io_bufs) as io_pool, \
             tc.tile_pool(name="mid", bufs=mid_bufs) as mid_pool, \
             tc.tile_pool(name="accs", bufs=1) as acc_pool, \
             tc.psum_pool(name="ps", bufs=1) as ps_pool:
            accSQ = acc_pool.tile([P, nt], F32)
            accG = acc_pool.tile([P, nt], F32)   # TS is_lt counts
            accSG = acc_pool.tile([P, nt], F32)  # Act Sign sums
            ones = acc_pool.tile([P, 1], F16)
            nc.gpsimd.memset(ones[:], 1.0)
            biasm = acc_pool.tile([P, 1], F32)
            nc.gpsimd.memset(biasm[:], -0.5)
            psrow = ps_pool.tile([1, MM], F32)
            nmm = C // MM
            for i in range(nt):
                X = io_pool.tile([P, 2 * C], F16, tag="X")
                Sf = io_pool.tile([P, C], F16, tag="Sf")
                nc.sync.dma_start(X[:], ab_d[:, i * 2 * C:(i + 1) * 2 * C])
                # software-DGE casting DMA: int8 {-1,+1} -> fp16
                nc.gpsimd.dma_start(Sf[:], s_d[:, i * C:(i + 1) * C])
                A = X[:, :C]
                B = X[:, C:]

                sq = mid_pool.tile([P, 2 * C], F16, tag="sq")
                d = mid_pool.tile([P, C], F16, tag="d")
                sd = mid_pool.tile([P, C], F16, tag="sd")
                w = mid_pool.tile([P, C], F16, tag="w")

                # Act: sum of (x-1/2)^2 over the whole tile
                nc.scalar.activation(sq[:], X[:], AF.Square, bias=biasm[:],
                                     accum_out=accSQ[:, i:i + 1])
                # DVE: d = b-a, sd = s*d (both 2x_1p)
                nc.vector.tensor_tensor(d[:], B, A, ALU.subtract)
                nc.vector.tensor_tensor(sd[:], Sf[:], d[:], ALU.mult)
                # PE: sum of sd via ones-matmul, accumulated across tiles
                for k in range(nmm):
                    nc.tensor.matmul(psrow[:], ones[:],
                                     sd[:, k * MM:(k + 1) * MM],
                                     start=(i == 0 and k == 0),
                                     stop=(i == nt - 1 and k == nmm - 1))
                if i < nt - N_SIGN_TILES:
                    # G on DVE: count(sd<0)
                    nc.vector.tensor_scalar(w[:], sd[:], 0.0, 0.0, ALU.is_lt,
                                            ALU.add,
                                            accum_out=accG[:, i:i + 1])
                else:
                    # G on Act: sum sign(sd); host converts to a count
                    nc.scalar.activation(w[:], sd[:], AF.Sign,
                                         accum_out=accSG[:, i:i + 1])

            rowsb = acc_pool.tile([1, MM], F32)
            nc.scalar.activation(rowsb[:], psrow[:], AF.Copy)
            nc.sync.dma_start(out_acc[:, 0 * nt:1 * nt], accSQ[:])
            nc.sync.dma_start(out_acc[:, 1 * nt:2 * nt], accG[:])
            nc.sync.dma_start(out_acc[:, 2 * nt:3 * nt], accSG[:])
            nc.sync.dma_start(out_row[:], rowsb[:])

    _split_excess_waits(nc, max_waits=1)
    return nc, nt


_CACHE = {}


def _get_program():
    if "prog" not in _CACHE:
        _CACHE["prog"] = _build()
    return _CACHE["prog"]


def kernel(pred, target):
    pred = np.asarray(pred)
    target = np.asarray(target)
    assert pred.shape == (N_TOTAL, 2) and pred.dtype == np.float32

    pred16 = pred.astype(np.float16)           # (N, 2) fp16
    s8 = (target.astype(np.int8) << 1) - 1     # {0,1} -> {-1,+1}

    nc, nt = _get_program()
    C = W2 // nt
    in_maps = []
    for c in range(N_CORES):
        # [P, nt, 2, C]: per tile an a-block then a b-block, both packed
        blk = pred16[c * R:(c + 1) * R].reshape(P, nt, C, 2)
        ab = np.ascontiguousarray(blk.transpose(0, 1, 3, 2)).reshape(P, 2 * W2)
        in_maps.append({
            "AB": ab,
            "S": s8[c * R:(c + 1) * R].reshape(P, W2),
        })

    res = run_bass_kernel_spmd(nc, in_maps, list(range(N_CORES)))

    sign_elems = N_SIGN_TILES * C * P
    total = 0.0
    for r in res.results:
        acc = np.asarray(r["out_acc"]).astype(np.float64)
        sSQ = acc[:, 0 * nt:1 * nt].sum()
        sG = acc[:, 1 * nt:2 * nt].sum()
        sSGN = acc[:, 2 * nt:3 * nt].sum()
        sSD = np.asarray(r["out_row"]).astype(np.float64).sum()
        g_total = sG + (sign_elems - sSGN) / 2.0
        total += sSQ + R / 2.0 - sSD + 2.0 * g_total
    return np.float32(total / N_TOTAL)
